# revision 10
# baseline (speedup 1.0000x reference)
"""CandidateFinder kernel for Trainium2 (8 NeuronCores, SPMD).

Problem: for each query i (per batch), find keys j where
  lsh_match(i,j) = any of 4 LSH hash buckets agree, AND
  trie_match(i,j) = all 12 sign bits of (batch -1) features agree.
Output [B, Sq, 64] int32: if count<=64, ascending candidate indices
right-aligned with -1 padding; if count>64, ascending top-64 by dot-sim.

Device strategy: the pair predicate is a matmul + threshold.
  - one-hot encode the 4 hash ids (4*32 = 128 dims) -> lshdot = #agreeing hashes
  - sign vectors sigma in {-1,+1}^12, query side scaled by 8 -> 8*triedot
  - total = lshdot + 8*triedot ; match <=> total >= 97 (exact integer logic,
    non-match max is 96: triedot=12 w/ lshdot=0, or 8*10+4=84 otherwise)
Each core: 1024 queries x 4096 keys. Per (key-tile 128, query-chunk 512):
two accumulating matmuls -> PSUM totals; DVE is_ge (0/1) or ACT Sign (+/-1)
threshold -> bf16 mask in SBUF; pack-matmul with 2^t weights compresses each
16-key group into one f32 bit-field. Host decodes bits -> candidate indices
(exact), right-aligns with -1 padding, and handles the (astronomically rare)
count>64 rows with an exact host fallback.

Sharding: batch x query-quarter across 8 cores (core c: batch c//4,
queries (c%4)*1024 ..+1024); full key set replicated per core.
"""

import numpy as np
from ml_dtypes import bfloat16

import concourse.bacc as bacc
import concourse.tile as tile
from concourse import mybir
from concourse.bass_utils import run_bass_kernel_spmd

B, S, D = 2, 4096, 12
H, BUCKETS, BW = 4, 32, 4.0
KMAX = 64
NCORES = 8
QPC = B * S // NCORES      # 1024 queries per core
NQC = QPC // 512           # 2 query chunks of 512
NKT = S // 128             # 32 key tiles
GROUP = 16                 # keys per packed bit-group
NG = 128 // GROUP          # 8 groups per key tile
FULL = (1 << GROUP) - 1    # 65535
THRESH = 96.5

TRACE = False              # set True (module flag) to capture an NTFF trace
LAST_RESULTS = None

_nc_cache = None


def _build():
    global _nc_cache
    if _nc_cache is not None:
        return _nc_cache
    nc = bacc.Bacc()
    bf16 = mybir.dt.bfloat16
    f32 = mybir.dt.float32

    ft_oh = nc.dram_tensor("ft_oh", [128, QPC], bf16, kind="ExternalInput")
    ft_sg = nc.dram_tensor("ft_sg", [D, QPC], bf16, kind="ExternalInput")
    gt_oh = nc.dram_tensor("gt_oh", [128, S], bf16, kind="ExternalInput")
    gt_sg = nc.dram_tensor("gt_sg", [D, S], bf16, kind="ExternalInput")
    out_d = nc.dram_tensor("out", [NQC, 8, 4, NG, 512], f32, kind="ExternalOutput")

    pack_np = np.zeros((128, NG), np.float32)
    for g in range(NG):
        for t in range(GROUP):
            pack_np[g * GROUP + t, g] = float(1 << t)
    pack_d = nc.inline_tensor(pack_np.astype(bfloat16), name="packw")
    bias_d = nc.inline_tensor(
        np.full((128, 1), -THRESH, np.float32), name="biasc"
    )

    with tile.TileContext(nc) as tc:
        with (
            tc.tile_pool(name="keys", bufs=1) as pool_k,
            tc.tile_pool(name="qrs", bufs=1) as pool_q,
            tc.tile_pool(name="mask", bufs=4) as pool_m,
            tc.tile_pool(name="ps_a", bufs=2, space="PSUM") as pool_pa,
            tc.tile_pool(name="ps_b", bufs=2, space="PSUM") as pool_pb,
            tc.tile_pool(name="outs", bufs=3) as pool_o,
            tc.tile_pool(name="cst", bufs=1) as pool_c,
        ):
            pack_t = pool_c.tile([128, NG], bf16, tag="pack")
            nc.sync.dma_start(out=pack_t[:], in_=pack_d[:])
            bias_t = pool_c.tile([128, 1], f32, tag="bias")
            nc.sync.dma_start(out=bias_t[:], in_=bias_d[:])

            g_oh = []
            for i in range(8):
                t_ = pool_k.tile([128, 512], bf16, tag=f"goh{i}")
                nc.sync.dma_start(out=t_[:], in_=gt_oh[:, i * 512:(i + 1) * 512])
                g_oh.append(t_)
            g_sg = pool_k.tile([D, S], bf16, tag="gsg")
            nc.sync.dma_start(out=g_sg[:], in_=gt_sg[:])

            f_oh, f_sg = [], []
            for qc in range(NQC):
                t1 = pool_q.tile([128, 512], bf16, tag=f"foh{qc}")
                nc.sync.dma_start(out=t1[:], in_=ft_oh[:, qc * 512:(qc + 1) * 512])
                f_oh.append(t1)
                t2 = pool_q.tile([D, 512], bf16, tag=f"fsg{qc}")
                nc.sync.dma_start(out=t2[:], in_=ft_sg[:, qc * 512:(qc + 1) * 512])
                f_sg.append(t2)

            for qc in range(NQC):
                for pb in range(8):
                    psB = pool_pb.tile([128, 512], f32)
                    for j in range(4):
                        kt = pb * 4 + j
                        psA = pool_pa.tile([128, 512], f32)
                        nc.tensor.matmul(
                            psA[:],
                            lhsT=g_oh[kt // 4][:, (kt % 4) * 128:(kt % 4 + 1) * 128],
                            rhs=f_oh[qc][:],
                            start=True, stop=False,
                        )
                        nc.tensor.matmul(
                            psA[:],
                            lhsT=g_sg[:, kt * 128:(kt + 1) * 128],
                            rhs=f_sg[qc][:],
                            start=False, stop=True,
                        )
                        m_ = pool_m.tile([128, 512], bf16)
                        if kt % 2 == 0:
                            nc.vector.tensor_scalar(
                                m_[:], psA[:], THRESH, None, mybir.AluOpType.is_ge
                            )
                        else:
                            nc.scalar.sign(m_[:], psA[:], bias=bias_t[:])
                        nc.tensor.matmul(
                            psB[j * 32:j * 32 + NG, :],
                            lhsT=pack_t[:], rhs=m_[:],
                            start=True, stop=True, tile_position=(0, j * 32),
                        )
                    sb = pool_o.tile([128, 512], f32)
                    if pb % 2 == 0:
                        nc.vector.tensor_copy(sb[:], psB[:])
                    else:
                        nc.scalar.copy(sb[:], psB[:])
                    for j in range(4):
                        nc.sync.dma_start(
                            out=out_d[qc, pb, j], in_=sb[j * 32:j * 32 + NG, :]
                        )

    nc.compile()  # wait legalization + reg alloc (bass2jax does not finalize)
    _nc_cache = nc
    return nc


def _hashes(x, proj):
    # mirror: floor((x @ lsh_proj) / BW).astype(int32) % BUCKETS
    d = x.astype(np.float32) @ proj.astype(np.float32)
    return np.floor(d / BW).astype(np.int32) % BUCKETS


def _prep(q, k, proj):
    qh = _hashes(q, proj)                       # [B,S,4]
    kh = _hashes(k, proj)
    rng = np.arange(BUCKETS, dtype=np.int32)
    q_oh = (qh[..., None] == rng).reshape(B, S, 128)
    k_oh = (kh[..., None] == rng).reshape(B, S, 128)
    sq = np.where(q[-1] > 0, np.float32(1.0), np.float32(-1.0))   # [S,12]
    sk = np.where(k[-1] > 0, np.float32(1.0), np.float32(-1.0))
    ftoh = np.ascontiguousarray(q_oh.astype(bfloat16).transpose(0, 2, 1))  # [B,128,S]
    gtoh = np.ascontiguousarray(k_oh.astype(bfloat16).transpose(0, 2, 1))
    ftsg = np.ascontiguousarray((8.0 * sq).astype(bfloat16).T)    # [12,S]
    gtsg = np.ascontiguousarray(sk.astype(bfloat16).T)
    return qh, kh, sq, sk, ftoh, gtoh, ftsg, gtsg


def _decode(raw):
    """raw: [8 cores, NQC, 8, 4, NG, 512] f32 -> (vi [B,S,NKT,NG] int64 bitfields)."""
    v = raw.reshape(2, 4, NQC, 8, 4, NG, 512)
    v = v.transpose(0, 1, 2, 6, 3, 4, 5)          # [b, quarter, qc, n, pb, j, g]
    v = np.ascontiguousarray(v).reshape(B, S, NKT, NG)
    vi = np.rint(v).astype(np.int64)
    odd = (np.arange(NKT) % 2) == 1               # ACT Sign tiles encode +/-1
    vi[:, :, odd, :] = (vi[:, :, odd, :] + FULL) // 2
    return vi


def _mask_row(b, i, qh, kh, sq, sk):
    lsh = (qh[b, i][None, :] == kh[b]).any(-1)                  # [S]
    trie = (sq[i][None, :] == sk).all(-1)                       # [S]
    return lsh & trie


def _topk_row(q, k, b, i, maskrow):
    sims = q[b, i].astype(np.float32) @ k[b].astype(np.float32).T
    vals = np.where(maskrow, sims, -np.inf)
    top = np.argsort(-vals, kind="stable")[:KMAX]               # jax top_k tiebreak
    return np.sort(top).astype(np.int32)


def _ensure_ntff_hook():
    """The container's antenv stub lacks axon_hooks; synthesize it from the
    boot module's ctypes NTFF helper so trace=True can capture HW timings."""
    import sys
    import types
    try:
        from antenv.axon_hooks import get_axon_ntff_profile_hook  # noqa: F401
        return
    except ImportError:
        pass
    from trn_agent_boot.trn_boot import _ntff_profile_via_ctypes
    hook = _ntff_profile_via_ctypes("/opt/axon/libaxon_pjrt.so")
    mod = types.ModuleType("antenv.axon_hooks")
    state = {"hook": hook}
    mod.get_axon_ntff_profile_hook = lambda: state["hook"]
    mod.set_axon_ntff_profile_hook = lambda h: state.update(hook=h)
    import antenv
    antenv.axon_hooks = mod
    sys.modules["antenv.axon_hooks"] = mod


def kernel(**inputs):
    global LAST_RESULTS
    q = np.asarray(inputs["query_features_up"], np.float32)
    k = np.asarray(inputs["key_features_up"], np.float32)
    proj = np.asarray(inputs["lsh_proj"], np.float32)

    qh, kh, sq, sk, ftoh, gtoh, ftsg, gtsg = _prep(q, k, proj)

    nc = _build()
    in_maps = []
    for c in range(NCORES):
        b = c // 4
        qoff = (c % 4) * QPC
        in_maps.append({
            "ft_oh": np.ascontiguousarray(ftoh[b][:, qoff:qoff + QPC]),
            "ft_sg": np.ascontiguousarray(ftsg[:, qoff:qoff + QPC]),
            "gt_oh": gtoh[b],
            "gt_sg": gtsg,
        })
    if TRACE:
        _ensure_ntff_hook()
    res = run_bass_kernel_spmd(
        nc, in_maps, core_ids=list(range(NCORES)), trace=TRACE
    )
    LAST_RESULTS = res
    raw = np.stack([r["out"].astype(np.float32) for r in res.results])

    vi = _decode(raw)

    # bitfields -> candidate triples (b, q, idx), ascending by construction order
    bq, qq, ktq, gq = np.nonzero(vi)
    vals = vi[bq, qq, ktq, gq]
    base = ktq * 128 + gq * GROUP
    cb, cq, ci = [], [], []
    for t in range(GROUP):
        selm = ((vals >> t) & 1).astype(bool)
        cb.append(bq[selm]); cq.append(qq[selm]); ci.append(base[selm] + t)
    cb = np.concatenate(cb); cq = np.concatenate(cq); ci = np.concatenate(ci)
    order = np.lexsort((ci, cq, cb))
    cb, cq, ci = cb[order], cq[order], ci[order]

    rowid = cb * S + cq
    counts = np.bincount(rowid, minlength=B * S)
    starts = np.concatenate(([0], np.cumsum(counts)))[:-1]
    ranks = np.arange(len(ci)) - starts[rowid]

    out = np.full((B * S, KMAX), -1, np.int32)
    cnt_row = counts[rowid]
    ok = cnt_row <= KMAX
    out[rowid[ok], (KMAX - cnt_row + ranks)[ok]] = ci[ok]

    # exact host fallback for count > KMAX rows (never happens in practice)
    for r in np.nonzero(counts > KMAX)[0]:
        b, i = divmod(int(r), S)
        mrow = _mask_row(b, i, qh, kh, sq, sk)
        out[r] = _topk_row(q, k, b, i, mrow)

    return out.reshape(B, S, KMAX)


# revision 12
# speedup vs baseline: 1.6653x; 1.6653x over previous
"""CandidateFinder kernel for Trainium2 (8 NeuronCores, SPMD).

Problem: for each query i (per batch), find keys j where
  lsh_match(i,j) = any of 4 LSH hash buckets agree, AND
  trie_match(i,j) = all 12 sign bits of (batch -1) features agree.
Output [B, Sq, 64] int32: if count<=64, ascending candidate indices
right-aligned with -1 padding; if count>64, ascending top-64 by dot-sim.

Device strategy: the pair predicate is a matmul + threshold.
  - one-hot encode the 4 hash ids (4*32 = 128 dims) -> lshdot = #agreeing hashes
  - trie part is batch-independent (signs always come from batch B-1), so each
    core handles 512 query INDICES x both batches and computes the trie
    threshold once per key tile:
      thr = 96.5 - 8*triedot   (features: [-8*sq x -? see _prep] + 96.5 bias)
      match <=> lshdot >= thr  (exact: lshdot in 0..4, thr in {0.5 +/- 8k})
  - per (key-tile, batch): one matmul -> lshdot PSUM; one DVE tensor_tensor
    is_ge against thr -> fp8 mask byte (0x38 iff match), staged 4 key tiles
    per SBUF tile and DMA'd out as raw bytes; host decodes bytes -> candidate
    indices (exact), right-aligns with -1 padding, and handles the
    (astronomically rare) count>64 rows with an exact host fallback.
"""

import numpy as np
from ml_dtypes import bfloat16, float8_e4m3

import concourse.bacc as bacc
import concourse.tile as tile
from concourse import mybir
from concourse.bass_utils import run_bass_kernel_spmd

B, S, D = 2, 4096, 12
H, BUCKETS, BW = 4, 32, 4.0
KMAX = 64
NCORES = 8
QPC = S // NCORES          # 512 query indices per core (x2 batches)
NKT = S // 128             # 32 key tiles
THRESH = 96.5
MATCH_BYTE = 0x38          # fp8e4 bit pattern of +1.0

TRACE = False              # set True (module flag) to capture an NTFF trace
LAST_RESULTS = None

_nc_cache = None


def _build():
    global _nc_cache
    if _nc_cache is not None:
        return _nc_cache
    nc = bacc.Bacc()
    bf16 = mybir.dt.bfloat16
    f8 = mybir.dt.float8e4
    f32 = mybir.dt.float32

    ft_oh = nc.dram_tensor("ft_oh", [2, 128, QPC], bf16, kind="ExternalInput")
    ft_sg = nc.dram_tensor("ft_sg", [D + 1, QPC], bf16, kind="ExternalInput")
    gt_oh = nc.dram_tensor("gt_oh", [2, 128, S], bf16, kind="ExternalInput")
    gt_sg = nc.dram_tensor("gt_sg", [D + 1, S], bf16, kind="ExternalInput")
    out_d = nc.dram_tensor("out", [2, NKT // 4, 128, 4 * QPC], f8,
                           kind="ExternalOutput")

    with tile.TileContext(nc) as tc:
        with (
            tc.tile_pool(name="keys", bufs=1) as pool_k,
            tc.tile_pool(name="qrs", bufs=1) as pool_q,
            tc.tile_pool(name="thr", bufs=3) as pool_t,
            tc.tile_pool(name="msk", bufs=2) as pool_m,
            tc.tile_pool(name="ps_t", bufs=2, space="PSUM") as pool_pt,
            tc.tile_pool(name="ps_a", bufs=3, space="PSUM") as pool_pa,
        ):
            g_oh = [[], []]
            for b in range(2):
                for i in range(8):
                    t_ = pool_k.tile([128, 512], bf16, tag=f"goh{b}_{i}")
                    nc.sync.dma_start(
                        out=t_[:], in_=gt_oh[b][:, i * 512:(i + 1) * 512])
                    g_oh[b].append(t_)
            g_sg = pool_k.tile([D + 1, S], bf16, tag="gsg")
            nc.sync.dma_start(out=g_sg[:], in_=gt_sg[:])

            f_oh = []
            for b in range(2):
                t1 = pool_q.tile([128, QPC], bf16, tag=f"foh{b}")
                nc.sync.dma_start(out=t1[:], in_=ft_oh[b])
                f_oh.append(t1)
            f_sg = pool_q.tile([D + 1, QPC], bf16, tag="fsg")
            nc.sync.dma_start(out=f_sg[:], in_=ft_sg[:])

            msk = [None, None]
            for kt in range(NKT):
                if kt % 4 == 0:
                    msk = [pool_m.tile([128, 4 * QPC], f8, tag=f"msk{b}",
                                       name=f"msk{b}_{kt}")
                           for b in range(2)]
                psT = pool_pt.tile([128, QPC], f32)
                nc.tensor.matmul(
                    psT[:],
                    lhsT=g_sg[:, kt * 128:(kt + 1) * 128],
                    rhs=f_sg[:],
                    start=True, stop=True,
                )
                thr = pool_t.tile([128, QPC], f32)
                nc.scalar.copy(thr[:], psT[:])
                for b in range(2):
                    psA = pool_pa.tile([128, QPC], f32)
                    nc.tensor.matmul(
                        psA[:],
                        lhsT=g_oh[b][kt // 4][:, (kt % 4) * 128:(kt % 4 + 1) * 128],
                        rhs=f_oh[b][:],
                        start=True, stop=True,
                    )
                    nc.vector.tensor_tensor(
                        msk[b][:, (kt % 4) * QPC:(kt % 4 + 1) * QPC],
                        psA[:], thr[:], mybir.AluOpType.is_ge,
                    )
                if kt % 4 == 3:
                    for b in range(2):
                        nc.sync.dma_start(out=out_d[b, kt // 4], in_=msk[b][:])

    nc.compile()  # wait legalization + reg alloc (bass2jax does not finalize)
    _nc_cache = nc
    return nc


def _hashes(x, proj):
    # mirror: floor((x @ lsh_proj) / BW).astype(int32) % BUCKETS
    d = x.astype(np.float32) @ proj.astype(np.float32)
    return np.floor(d / BW).astype(np.int32) % BUCKETS


def _prep(q, k, proj):
    qh = _hashes(q, proj)                       # [B,S,4]
    kh = _hashes(k, proj)
    rng = np.arange(BUCKETS, dtype=np.int32)
    q_oh = (qh[..., None] == rng).reshape(B, S, 128)
    k_oh = (kh[..., None] == rng).reshape(B, S, 128)
    sq = np.where(q[-1] > 0, np.float32(1.0), np.float32(-1.0))   # [S,12]
    sk = np.where(k[-1] > 0, np.float32(1.0), np.float32(-1.0))
    ftoh = np.ascontiguousarray(q_oh.astype(bfloat16).transpose(0, 2, 1))  # [B,128,S]
    gtoh = np.ascontiguousarray(k_oh.astype(bfloat16).transpose(0, 2, 1))
    # thr matmul: thr = 96.5 - 8*(sq . sk)  ->  F = [8*sq, 1], G = [-sk, 96.5]
    ftsg = np.concatenate([8.0 * sq.T, np.ones((1, S), np.float32)], 0)
    gtsg = np.concatenate([-sk.T, np.full((1, S), THRESH, np.float32)], 0)
    return (qh, kh, sq, sk, ftoh, gtoh,
            ftsg.astype(bfloat16), gtsg.astype(bfloat16))


def _mask_row(b, i, qh, kh, sq, sk):
    lsh = (qh[b, i][None, :] == kh[b]).any(-1)                  # [S]
    trie = (sq[i][None, :] == sk).all(-1)                       # [S]
    return lsh & trie


def _topk_row(q, k, b, i, maskrow):
    sims = q[b, i].astype(np.float32) @ k[b].astype(np.float32).T
    vals = np.where(maskrow, sims, -np.inf)
    top = np.argsort(-vals, kind="stable")[:KMAX]               # jax top_k tiebreak
    return np.sort(top).astype(np.int32)


def _ensure_ntff_hook():
    """The container's antenv stub lacks axon_hooks; synthesize it from the
    boot module's ctypes NTFF helper so trace=True can capture HW timings."""
    import sys
    import types
    try:
        from antenv.axon_hooks import get_axon_ntff_profile_hook  # noqa: F401
        return
    except ImportError:
        pass
    from trn_agent_boot.trn_boot import _ntff_profile_via_ctypes
    hook = _ntff_profile_via_ctypes("/opt/axon/libaxon_pjrt.so")
    mod = types.ModuleType("antenv.axon_hooks")
    state = {"hook": hook}
    mod.get_axon_ntff_profile_hook = lambda: state["hook"]
    mod.set_axon_ntff_profile_hook = lambda h: state.update(hook=h)
    import antenv
    antenv.axon_hooks = mod
    sys.modules["antenv.axon_hooks"] = mod


def kernel(**inputs):
    global LAST_RESULTS
    q = np.asarray(inputs["query_features_up"], np.float32)
    k = np.asarray(inputs["key_features_up"], np.float32)
    proj = np.asarray(inputs["lsh_proj"], np.float32)

    qh, kh, sq, sk, ftoh, gtoh, ftsg, gtsg = _prep(q, k, proj)

    nc = _build()
    in_maps = []
    for c in range(NCORES):
        qoff = c * QPC
        in_maps.append({
            "ft_oh": np.ascontiguousarray(ftoh[:, :, qoff:qoff + QPC]),
            "ft_sg": np.ascontiguousarray(ftsg[:, qoff:qoff + QPC]),
            "gt_oh": gtoh,
            "gt_sg": gtsg,
        })
    if TRACE:
        _ensure_ntff_hook()
    res = run_bass_kernel_spmd(
        nc, in_maps, core_ids=list(range(NCORES)), trace=TRACE
    )
    LAST_RESULTS = res

    # raw mask bytes -> bool match grid [B, Sq, Sk]
    match = np.empty((B, S, S), np.bool_)
    for c in range(NCORES):
        raw = res.results[c]["out"].view(np.uint8)   # [2, 8, 128, 4*QPC]
        raw = raw.reshape(2, 8, 128, 4, QPC)         # [b, g8, p, j, n]
        # key = (g8*4 + j)*128 + p ; query = c*QPC + n
        m = (raw == MATCH_BYTE).transpose(0, 4, 1, 3, 2)  # [b, n, g8, j, p]
        match[:, c * QPC:(c + 1) * QPC, :] = m.reshape(2, QPC, S)

    cb, cq, ci = np.nonzero(match)
    rowid = cb.astype(np.int64) * S + cq
    counts = np.bincount(rowid, minlength=B * S)
    starts = np.concatenate(([0], np.cumsum(counts)))[:-1]
    ranks = np.arange(len(ci)) - starts[rowid]

    out = np.full((B * S, KMAX), -1, np.int32)
    cnt_row = counts[rowid]
    ok = cnt_row <= KMAX
    out[rowid[ok], (KMAX - cnt_row + ranks)[ok]] = ci[ok]

    # exact host fallback for count > KMAX rows (never happens in practice)
    for r in np.nonzero(counts > KMAX)[0]:
        b, i = divmod(int(r), S)
        mrow = _mask_row(b, i, qh, kh, sq, sk)
        out[r] = _topk_row(q, k, b, i, mrow)

    return out.reshape(B, S, KMAX)


# revision 13
# speedup vs baseline: 1.7225x; 1.0343x over previous
"""CandidateFinder kernel for Trainium2 (8 NeuronCores, SPMD).

Problem: for each query i (per batch), find keys j where
  lsh_match(i,j) = any of 4 LSH hash buckets agree, AND
  trie_match(i,j) = all 12 sign bits of (batch -1) features agree.
Output [B, Sq, 64] int32: if count<=64, ascending candidate indices
right-aligned with -1 padding; if count>64, ascending top-64 by dot-sim.

Device strategy (v3): the pair predicate is a matmul + threshold.
  - one-hot encode the 4 hash ids (4*32 = 128 dims) -> lshdot = #agreeing hashes
  - trie part is batch-independent (signs always come from batch B-1), so each
    core handles 512 query INDICES x both batches and computes the trie
    threshold once per key tile: thr = 96.5 - 8*triedot
      match <=> lshdot >= thr  (exact integer+half logic)
  - per key tile: one K=13 matmul -> thr PSUM -> ACT copy to SBUF; two K=128
    matmuls (one per batch) -> [128,1024] PSUM; one DVE tensor_tensor is_ge
    with 0-step-broadcast thr -> fp8 mask bytes (0x38 iff match); 4 key tiles
    staged per SBUF tile, 8 big DMAs ship raw bytes. Host decodes bytes ->
    candidate indices (exact), right-aligns with -1 padding, and handles the
    (astronomically rare) count>64 rows with an exact host fallback.
"""

import copy

import numpy as np
from ml_dtypes import bfloat16, float8_e4m3

import bass_rust
import concourse.bacc as bacc
import concourse.tile as tile
from concourse import mybir
from concourse.bass_utils import run_bass_kernel_spmd

B, S, D = 2, 4096, 12
H, BUCKETS, BW = 4, 32, 4.0
KMAX = 64
NCORES = 8
QPC = S // NCORES          # 512 query indices per core (x2 batches)
NKT = S // 128             # 32 key tiles
THRESH = 96.5
MATCH_BYTE = 0x38          # fp8e4 bit pattern of +1.0

TRACE = False              # set True (module flag) to capture an NTFF trace
LAST_RESULTS = None

_nc_cache = None


def _bcast2(ap):
    """Insert a 0-step [*, 2] dim after the partition dim (free broadcast)."""
    b = copy.copy(ap)
    b.ap = bass_rust.VecI64Pair([list(ap.ap[0]), [0, 2], list(ap.ap[1])])
    return b


def _build():
    global _nc_cache
    if _nc_cache is not None:
        return _nc_cache
    nc = bacc.Bacc()
    bf16 = mybir.dt.bfloat16
    f8 = mybir.dt.float8e4
    f32 = mybir.dt.float32

    ft_oh = nc.dram_tensor("ft_oh", [2, 128, QPC], bf16, kind="ExternalInput")
    ft_sg = nc.dram_tensor("ft_sg", [D + 1, QPC], bf16, kind="ExternalInput")
    gt_oh = nc.dram_tensor("gt_oh", [2, 128, S], bf16, kind="ExternalInput")
    gt_sg = nc.dram_tensor("gt_sg", [D + 1, S], bf16, kind="ExternalInput")
    # [g8, key-in-tile, j, batch, query]
    out_d = nc.dram_tensor("out", [NKT // 4, 128, 4, 2, QPC], f8,
                           kind="ExternalOutput")

    with tile.TileContext(nc) as tc:
        with (
            tc.tile_pool(name="keys", bufs=1) as pool_k,
            tc.tile_pool(name="qrs", bufs=1) as pool_q,
            tc.tile_pool(name="thr", bufs=3) as pool_t,
            tc.tile_pool(name="msk", bufs=2) as pool_m,
            tc.tile_pool(name="ps_t", bufs=2, space="PSUM") as pool_pt,
            tc.tile_pool(name="ps_a", bufs=2, space="PSUM") as pool_pa,
        ):
            # loads ordered so key-tile 0 dependencies land first; bulk key
            # one-hots go through SWDGE (gpsimd) to parallelize trigger issue
            f_sg = pool_q.tile([D + 1, QPC], bf16, tag="fsg")
            nc.sync.dma_start(out=f_sg[:], in_=ft_sg[:])
            g_sg = []
            for i in range(8):
                t_ = pool_k.tile([D + 1, 512], bf16, tag=f"gsg{i}")
                nc.sync.dma_start(out=t_[:], in_=gt_sg[:, i * 512:(i + 1) * 512])
                g_sg.append(t_)
            f_oh = []
            for b in range(2):
                t1 = pool_q.tile([128, QPC], bf16, tag=f"foh{b}")
                nc.sync.dma_start(out=t1[:], in_=ft_oh[b])
                f_oh.append(t1)
            g_oh = [[], []]
            for i in range(8):
                for b in range(2):
                    t_ = pool_k.tile([128, 512], bf16, tag=f"goh{b}_{i}")
                    nc.gpsimd.dma_start(
                        out=t_[:], in_=gt_oh[b][:, i * 512:(i + 1) * 512])
                    g_oh[b].append(t_)

            msk = None
            for kt in range(NKT):
                if kt % 4 == 0:
                    msk = pool_m.tile([128, 4 * 2 * QPC], f8, tag="msk",
                                      name=f"msk_{kt}")
                psT = pool_pt.tile([128, QPC], f32)
                nc.tensor.matmul(
                    psT[:],
                    lhsT=g_sg[kt // 4][:, (kt % 4) * 128:(kt % 4 + 1) * 128],
                    rhs=f_sg[:],
                    start=True, stop=True,
                )
                thr = pool_t.tile([128, QPC], f32)
                nc.scalar.copy(thr[:], psT[:])
                psA = pool_pa.tile([128, 2 * QPC], f32)
                for b in range(2):
                    nc.tensor.matmul(
                        psA[:, b * QPC:(b + 1) * QPC],
                        lhsT=g_oh[b][kt // 4][:, (kt % 4) * 128:(kt % 4 + 1) * 128],
                        rhs=f_oh[b][:],
                        start=True, stop=True,
                    )
                nc.vector.tensor_tensor(
                    msk[:, (kt % 4) * 1024:(kt % 4 + 1) * 1024]
                        .rearrange("p (b n) -> p b n", b=2),
                    psA[:].rearrange("p (b n) -> p b n", b=2),
                    _bcast2(thr[:]),
                    mybir.AluOpType.is_ge,
                )
                if kt % 4 == 3:
                    nc.sync.dma_start(out=out_d[kt // 4], in_=msk[:])

    nc.compile()  # wait legalization + reg alloc (bass2jax does not finalize)
    _nc_cache = nc
    return nc


def _hashes(x, proj):
    # mirror: floor((x @ lsh_proj) / BW).astype(int32) % BUCKETS
    d = x.astype(np.float32) @ proj.astype(np.float32)
    return np.floor(d / BW).astype(np.int32) % BUCKETS


def _prep(q, k, proj):
    qh = _hashes(q, proj)                       # [B,S,4]
    kh = _hashes(k, proj)
    rng = np.arange(BUCKETS, dtype=np.int32)
    q_oh = (qh[..., None] == rng).reshape(B, S, 128)
    k_oh = (kh[..., None] == rng).reshape(B, S, 128)
    sq = np.where(q[-1] > 0, np.float32(1.0), np.float32(-1.0))   # [S,12]
    sk = np.where(k[-1] > 0, np.float32(1.0), np.float32(-1.0))
    ftoh = np.ascontiguousarray(q_oh.astype(bfloat16).transpose(0, 2, 1))  # [B,128,S]
    gtoh = np.ascontiguousarray(k_oh.astype(bfloat16).transpose(0, 2, 1))
    # thr matmul: thr = 96.5 - 8*(sq . sk)  ->  F = [8*sq, 1], G = [-sk, 96.5]
    ftsg = np.concatenate([8.0 * sq.T, np.ones((1, S), np.float32)], 0)
    gtsg = np.concatenate([-sk.T, np.full((1, S), THRESH, np.float32)], 0)
    return (qh, kh, sq, sk, ftoh, gtoh,
            ftsg.astype(bfloat16), gtsg.astype(bfloat16))


def _mask_row(b, i, qh, kh, sq, sk):
    lsh = (qh[b, i][None, :] == kh[b]).any(-1)                  # [S]
    trie = (sq[i][None, :] == sk).all(-1)                       # [S]
    return lsh & trie


def _topk_row(q, k, b, i, maskrow):
    sims = q[b, i].astype(np.float32) @ k[b].astype(np.float32).T
    vals = np.where(maskrow, sims, -np.inf)
    top = np.argsort(-vals, kind="stable")[:KMAX]               # jax top_k tiebreak
    return np.sort(top).astype(np.int32)


def _ensure_ntff_hook():
    """The container's antenv stub lacks axon_hooks; synthesize it from the
    boot module's ctypes NTFF helper so trace=True can capture HW timings."""
    import sys
    import types
    try:
        from antenv.axon_hooks import get_axon_ntff_profile_hook  # noqa: F401
        return
    except ImportError:
        pass
    from trn_agent_boot.trn_boot import _ntff_profile_via_ctypes
    hook = _ntff_profile_via_ctypes("/opt/axon/libaxon_pjrt.so")
    mod = types.ModuleType("antenv.axon_hooks")
    state = {"hook": hook}
    mod.get_axon_ntff_profile_hook = lambda: state["hook"]
    mod.set_axon_ntff_profile_hook = lambda h: state.update(hook=h)
    import antenv
    antenv.axon_hooks = mod
    sys.modules["antenv.axon_hooks"] = mod


def kernel(**inputs):
    global LAST_RESULTS
    q = np.asarray(inputs["query_features_up"], np.float32)
    k = np.asarray(inputs["key_features_up"], np.float32)
    proj = np.asarray(inputs["lsh_proj"], np.float32)

    qh, kh, sq, sk, ftoh, gtoh, ftsg, gtsg = _prep(q, k, proj)

    nc = _build()
    in_maps = []
    for c in range(NCORES):
        qoff = c * QPC
        in_maps.append({
            "ft_oh": np.ascontiguousarray(ftoh[:, :, qoff:qoff + QPC]),
            "ft_sg": np.ascontiguousarray(ftsg[:, qoff:qoff + QPC]),
            "gt_oh": gtoh,
            "gt_sg": gtsg,
        })
    if TRACE:
        _ensure_ntff_hook()
    res = run_bass_kernel_spmd(
        nc, in_maps, core_ids=list(range(NCORES)), trace=TRACE
    )
    LAST_RESULTS = res

    # raw mask bytes -> bool match grid [B, Sq, Sk]
    match = np.empty((B, S, S), np.bool_)
    for c in range(NCORES):
        raw = res.results[c]["out"].view(np.uint8)   # [8, 128, 4, 2, QPC]
        # key = (g8*4 + j)*128 + p ; query = c*QPC + n
        m = (raw == MATCH_BYTE).transpose(3, 4, 0, 2, 1)  # [b, n, g8, j, p]
        match[:, c * QPC:(c + 1) * QPC, :] = m.reshape(2, QPC, S)

    cb, cq, ci = np.nonzero(match)
    rowid = cb.astype(np.int64) * S + cq
    counts = np.bincount(rowid, minlength=B * S)
    starts = np.concatenate(([0], np.cumsum(counts)))[:-1]
    ranks = np.arange(len(ci)) - starts[rowid]

    out = np.full((B * S, KMAX), -1, np.int32)
    cnt_row = counts[rowid]
    ok = cnt_row <= KMAX
    out[rowid[ok], (KMAX - cnt_row + ranks)[ok]] = ci[ok]

    # exact host fallback for count > KMAX rows (never happens in practice)
    for r in np.nonzero(counts > KMAX)[0]:
        b, i = divmod(int(r), S)
        mrow = _mask_row(b, i, qh, kh, sq, sk)
        out[r] = _topk_row(q, k, b, i, mrow)

    return out.reshape(B, S, KMAX)


# revision 14
# speedup vs baseline: 1.7953x; 1.0422x over previous
"""CandidateFinder kernel for Trainium2 (8 NeuronCores, SPMD).

Problem: for each query i (per batch), find keys j where
  lsh_match(i,j) = any of 4 LSH hash buckets agree, AND
  trie_match(i,j) = all 12 sign bits of (batch -1) features agree.
Output [B, Sq, 64] int32: if count<=64, ascending candidate indices
right-aligned with -1 padding; if count>64, ascending top-64 by dot-sim.

Device strategy (v3): the pair predicate is a matmul + threshold.
  - one-hot encode the 4 hash ids (4*32 = 128 dims) -> lshdot = #agreeing hashes
  - trie part is batch-independent (signs always come from batch B-1), so each
    core handles 512 query INDICES x both batches and computes the trie
    threshold once per key tile: thr = 96.5 - 8*triedot
      match <=> lshdot >= thr  (exact integer+half logic)
  - per key tile: one K=13 matmul -> thr PSUM -> ACT copy to SBUF; two K=128
    matmuls (one per batch) -> [128,1024] PSUM; one DVE tensor_tensor is_ge
    with 0-step-broadcast thr -> fp8 mask bytes (0x38 iff match); 4 key tiles
    staged per SBUF tile, 8 big DMAs ship raw bytes. Host decodes bytes ->
    candidate indices (exact), right-aligns with -1 padding, and handles the
    (astronomically rare) count>64 rows with an exact host fallback.
"""

import copy

import numpy as np
from ml_dtypes import bfloat16, float8_e4m3

import bass_rust
import concourse.bacc as bacc
import concourse.tile as tile
from concourse import mybir
from concourse.bass_utils import run_bass_kernel_spmd

B, S, D = 2, 4096, 12
H, BUCKETS, BW = 4, 32, 4.0
KMAX = 64
NCORES = 8
QPC = S // NCORES          # 512 query indices per core (x2 batches)
NKT = S // 128             # 32 key tiles
THRESH = 96.5
MATCH_BYTE = 0x38          # fp8e4 bit pattern of +1.0

TRACE = False              # set True (module flag) to capture an NTFF trace
LAST_RESULTS = None

_nc_cache = None


def _bcast2(ap):
    """Insert a 0-step [*, 2] dim after the partition dim (free broadcast)."""
    b = copy.copy(ap)
    b.ap = bass_rust.VecI64Pair([list(ap.ap[0]), [0, 2], list(ap.ap[1])])
    return b


def _build():
    global _nc_cache
    if _nc_cache is not None:
        return _nc_cache
    nc = bacc.Bacc()
    bf16 = mybir.dt.bfloat16
    f8 = mybir.dt.float8e4
    f32 = mybir.dt.float32

    ft_oh = nc.dram_tensor("ft_oh", [2, 128, QPC], f8, kind="ExternalInput")
    ft_sg = nc.dram_tensor("ft_sg", [D + 2, QPC], f8, kind="ExternalInput")
    gt_oh = nc.dram_tensor("gt_oh", [2, 128, S], f8, kind="ExternalInput")
    gt_sg = nc.dram_tensor("gt_sg", [D + 2, S], f8, kind="ExternalInput")
    # [g8, key-in-tile, j, batch, query]
    out_d = nc.dram_tensor("out", [NKT // 4, 128, 4, 2, QPC], f8,
                           kind="ExternalOutput")

    with tile.TileContext(nc) as tc:
        with (
            tc.tile_pool(name="keys", bufs=1) as pool_k,
            tc.tile_pool(name="qrs", bufs=1) as pool_q,
            tc.tile_pool(name="thr", bufs=4) as pool_t,
            tc.tile_pool(name="msk", bufs=3) as pool_m,
            tc.tile_pool(name="ps_t", bufs=2, space="PSUM") as pool_pt,
            tc.tile_pool(name="ps_a", bufs=2, space="PSUM") as pool_pa,
        ):
            # loads ordered so key-tile 0 dependencies land first; bulk key
            # one-hots go through SWDGE (gpsimd) to parallelize trigger issue
            f_sg = pool_q.tile([D + 2, QPC], f8, tag="fsg")
            nc.sync.dma_start(out=f_sg[:], in_=ft_sg[:])
            g_sg = []
            for i in range(8):
                t_ = pool_k.tile([D + 2, 512], f8, tag=f"gsg{i}")
                nc.sync.dma_start(out=t_[:], in_=gt_sg[:, i * 512:(i + 1) * 512])
                g_sg.append(t_)
            f_oh = []
            for b in range(2):
                t1 = pool_q.tile([128, QPC], f8, tag=f"foh{b}")
                nc.sync.dma_start(out=t1[:], in_=ft_oh[b])
                f_oh.append(t1)
            g_oh = [[], []]
            for i in range(8):
                for b in range(2):
                    t_ = pool_k.tile([128, 512], f8, tag=f"goh{b}_{i}")
                    nc.gpsimd.dma_start(
                        out=t_[:], in_=gt_oh[b][:, i * 512:(i + 1) * 512])
                    g_oh[b].append(t_)

            msk = None
            for kt in range(NKT):
                if kt % 4 == 0:
                    msk = pool_m.tile([128, 4 * 2 * QPC], f8, tag="msk",
                                      name=f"msk_{kt}")
                psT = pool_pt.tile([128, QPC], f32)
                nc.tensor.matmul(
                    psT[:],
                    lhsT=g_sg[kt // 4][:, (kt % 4) * 128:(kt % 4 + 1) * 128],
                    rhs=f_sg[:],
                    start=True, stop=True,
                )
                thr = pool_t.tile([128, QPC], f32)
                nc.scalar.copy(thr[:], psT[:])
                psA = pool_pa.tile([128, 2 * QPC], f32)
                for b in range(2):
                    nc.tensor.matmul(
                        psA[:, b * QPC:(b + 1) * QPC],
                        lhsT=g_oh[b][kt // 4][:, (kt % 4) * 128:(kt % 4 + 1) * 128],
                        rhs=f_oh[b][:],
                        start=True, stop=True,
                    )
                nc.vector.tensor_tensor(
                    msk[:, (kt % 4) * 1024:(kt % 4 + 1) * 1024]
                        .rearrange("p (b n) -> p b n", b=2),
                    psA[:].rearrange("p (b n) -> p b n", b=2),
                    _bcast2(thr[:]),
                    mybir.AluOpType.is_ge,
                )
                if kt % 4 == 3:
                    nc.sync.dma_start(out=out_d[kt // 4], in_=msk[:])

    nc.compile()  # wait legalization + reg alloc (bass2jax does not finalize)
    _nc_cache = nc
    return nc


def _hashes(x, proj):
    # mirror: floor((x @ lsh_proj) / BW).astype(int32) % BUCKETS
    d = x.astype(np.float32) @ proj.astype(np.float32)
    return np.floor(d / BW).astype(np.int32) % BUCKETS


def _prep(q, k, proj):
    qh = _hashes(q, proj)                       # [B,S,4]
    kh = _hashes(k, proj)
    rng = np.arange(BUCKETS, dtype=np.int32)
    q_oh = (qh[..., None] == rng).reshape(B, S, 128)
    k_oh = (kh[..., None] == rng).reshape(B, S, 128)
    sq = np.where(q[-1] > 0, np.float32(1.0), np.float32(-1.0))   # [S,12]
    sk = np.where(k[-1] > 0, np.float32(1.0), np.float32(-1.0))
    ftoh = np.ascontiguousarray(q_oh.astype(float8_e4m3).transpose(0, 2, 1))  # [B,128,S]
    gtoh = np.ascontiguousarray(k_oh.astype(float8_e4m3).transpose(0, 2, 1))
    # thr matmul: thr = 96.5 - 8*(sq . sk); 96.5 is not fp8-exact so the bias
    # row splits into 96 + 0.5: F = [8*sq, 1, 1], G = [-sk, 96, 0.5]
    ones = np.ones((1, S), np.float32)
    ftsg = np.concatenate([8.0 * sq.T, ones, ones], 0)
    gtsg = np.concatenate([-sk.T, 96.0 * ones, 0.5 * ones], 0)
    return (qh, kh, sq, sk, ftoh, gtoh,
            ftsg.astype(float8_e4m3), gtsg.astype(float8_e4m3))


def _mask_row(b, i, qh, kh, sq, sk):
    lsh = (qh[b, i][None, :] == kh[b]).any(-1)                  # [S]
    trie = (sq[i][None, :] == sk).all(-1)                       # [S]
    return lsh & trie


def _topk_row(q, k, b, i, maskrow):
    sims = q[b, i].astype(np.float32) @ k[b].astype(np.float32).T
    vals = np.where(maskrow, sims, -np.inf)
    top = np.argsort(-vals, kind="stable")[:KMAX]               # jax top_k tiebreak
    return np.sort(top).astype(np.int32)


def _ensure_ntff_hook():
    """The container's antenv stub lacks axon_hooks; synthesize it from the
    boot module's ctypes NTFF helper so trace=True can capture HW timings."""
    import sys
    import types
    try:
        from antenv.axon_hooks import get_axon_ntff_profile_hook  # noqa: F401
        return
    except ImportError:
        pass
    from trn_agent_boot.trn_boot import _ntff_profile_via_ctypes
    hook = _ntff_profile_via_ctypes("/opt/axon/libaxon_pjrt.so")
    mod = types.ModuleType("antenv.axon_hooks")
    state = {"hook": hook}
    mod.get_axon_ntff_profile_hook = lambda: state["hook"]
    mod.set_axon_ntff_profile_hook = lambda h: state.update(hook=h)
    import antenv
    antenv.axon_hooks = mod
    sys.modules["antenv.axon_hooks"] = mod


def kernel(**inputs):
    global LAST_RESULTS
    q = np.asarray(inputs["query_features_up"], np.float32)
    k = np.asarray(inputs["key_features_up"], np.float32)
    proj = np.asarray(inputs["lsh_proj"], np.float32)

    qh, kh, sq, sk, ftoh, gtoh, ftsg, gtsg = _prep(q, k, proj)

    nc = _build()
    in_maps = []
    for c in range(NCORES):
        qoff = c * QPC
        in_maps.append({
            "ft_oh": np.ascontiguousarray(ftoh[:, :, qoff:qoff + QPC]),
            "ft_sg": np.ascontiguousarray(ftsg[:, qoff:qoff + QPC]),
            "gt_oh": gtoh,
            "gt_sg": gtsg,
        })
    if TRACE:
        _ensure_ntff_hook()
    res = run_bass_kernel_spmd(
        nc, in_maps, core_ids=list(range(NCORES)), trace=TRACE
    )
    LAST_RESULTS = res

    # raw mask bytes -> bool match grid [B, Sq, Sk]
    match = np.empty((B, S, S), np.bool_)
    for c in range(NCORES):
        raw = res.results[c]["out"].view(np.uint8)   # [8, 128, 4, 2, QPC]
        # key = (g8*4 + j)*128 + p ; query = c*QPC + n
        m = (raw == MATCH_BYTE).transpose(3, 4, 0, 2, 1)  # [b, n, g8, j, p]
        match[:, c * QPC:(c + 1) * QPC, :] = m.reshape(2, QPC, S)

    cb, cq, ci = np.nonzero(match)
    rowid = cb.astype(np.int64) * S + cq
    counts = np.bincount(rowid, minlength=B * S)
    starts = np.concatenate(([0], np.cumsum(counts)))[:-1]
    ranks = np.arange(len(ci)) - starts[rowid]

    out = np.full((B * S, KMAX), -1, np.int32)
    cnt_row = counts[rowid]
    ok = cnt_row <= KMAX
    out[rowid[ok], (KMAX - cnt_row + ranks)[ok]] = ci[ok]

    # exact host fallback for count > KMAX rows (never happens in practice)
    for r in np.nonzero(counts > KMAX)[0]:
        b, i = divmod(int(r), S)
        mrow = _mask_row(b, i, qh, kh, sq, sk)
        out[r] = _topk_row(q, k, b, i, mrow)

    return out.reshape(B, S, KMAX)
